# revision 13
# baseline (speedup 1.0000x reference)
"""ConcatCritic MLP on 8 Trainium2 NeuronCores.

Computes out[a, b] = f(concat(x[a], y[b])) for a tiny 4-layer MLP
(256->128->64->8->1 with ReLU), i.e. a [1024, 1024] score matrix.

Sharding (per spec hint): x's batch dim across the 8 cores (128 rows each);
y and the weights replicated. Each core computes a [128, 1024] output block.

Dataflow per core (feature-on-partition layout):
  - Split layer 1: concat(x,y) @ W1 = x @ W1[:128] + y @ W1[128:].
    xab[f, i] = (W1a^T @ x^T)[f, i] + b1[f]   (128 x 128, fp32)
    ybT[f, j] = (W1b^T @ y^T)[f, j]           (128 x 1024, bf16)
  - h1_i = relu(ybT + xab[:, i])  -- one DVE tensor_scalar per i (bf16, 4x)
  - L2: PE matmul, stationary W2 [128, 64]; even i -> PSUM rows 0:64
    (tile_position (0,0)), odd i -> rows 64:128 ((0,64)); one PSUM tile
    holds a pair of i's -> relu(+b2) evacuates [128, 1024] at once.
  - L3: stationary [128, 64] zero-padded 16-col strips (variant v for pair
    p = 4t+v) accumulate 4 pairs into each 64-row PSUM half; 8 pairs fill a
    dense [128, 1024] "h3pack" (16 i's) -> relu(+b3) evacuates at once.
  - L4: stationary [128, 64] with one W4 entry per (row-block, i) strip;
    8 groups accumulate into one [128, 1024] PSUM = the core's full output
    block (+b4 on evacuation).

All matmuls are bf16 (1 cycle/col, and fp32 self-loading matmuls overflow
the LDWEIGHTS sync-wait capacity in walrus) in 128x64 column-tiling mode
(tile_positions (0,0)/(0,64) only) so the PE never mode-switches. PSUM
accumulation is fp32. Inputs arrive as two packed tensors (one fp32, one
bf16) so consumers wait on at most one DMA queue each.
"""

import numpy as np
import ml_dtypes

import concourse.bass as bass
import concourse.bacc as bacc
import concourse.mybir as mybir
import concourse.tile as tile
from concourse.bass_utils import run_bass_kernel_spmd

BF16 = ml_dtypes.bfloat16
N_CORES = 8
B = 1024
D = 128
NI = B // N_CORES  # 128 rows of x per core
J = B              # full y batch per core
JC = 512           # matmul free-dim chunk (one PSUM bank)

# packed fp32 input layout (columns)
F_XT = 0            # [128, 128] x^T shard
F_YT = F_XT + NI    # [128, 1024] y^T
F_B1 = F_YT + J     # [128, 1] b1
F_B2 = F_B1 + 1     # [128, 1] b2 tiled x2
F_B3 = F_B2 + 1     # [128, 1] b3 tiled x16
F_B4 = F_B3 + 1     # [128, 1] b4 broadcast
F_TOT = F_B4 + 1

# packed bf16 input layout (columns)
H_W1A = 0             # [128, 128]
H_W1B = H_W1A + D     # [128, 128]
H_W2 = H_W1B + D      # [128, 64]
H_W3P = H_W2 + 64     # [128, 4*64]
H_W4P = H_W3P + 256   # [128, 4*64]
H_TOT = H_W4P + 256

# h2 evacuation engine: VectorE for global pair indices where
# (gp % EVAC_DVE_MOD) == EVAC_DVE_PHASE, else ScalarE. Spacing the DVE
# evacs breaks up consecutive ScalarE evacuations (which otherwise gate
# the ps2 slot turnaround and stall the PE) while keeping the DVE — which
# also produces every h1 — from becoming the bottleneck.
EVAC_DVE_MOD = 5
EVAC_DVE_PHASE = -1  # -1: all evacuations on ScalarE

_CACHE = {}


def _i_local_of_row(r):
    # h3pack row r -> which of the group's 16 i's it holds
    t, v, b = r // 64, (r % 64) // 16, (r % 16) // 8
    return 2 * (4 * t + v) + b


def _build_packed_weights(W3, W4):
    W3P = np.zeros((4, 128, 64), np.float32)
    for v in range(4):
        W3P[v, 0:64, 16 * v : 16 * v + 8] = W3
        W3P[v, 64:128, 16 * v + 8 : 16 * v + 16] = W3
    W4P = np.zeros((4, 128, 64), np.float32)
    for v4 in range(4):
        for r in range(128):
            c = 16 * v4 + _i_local_of_row(r)
            W4P[v4, r, c] = W4[r % 8, 0]
    return W3P, W4P


def _build_bass():
    nc = bacc.Bacc("TRN2", target_bir_lowering=False)
    f32 = mybir.dt.float32
    bf16 = mybir.dt.bfloat16

    f32in = nc.dram_tensor("f32in", [D, F_TOT], f32, kind="ExternalInput")
    bf16in = nc.dram_tensor("bf16in", [D, H_TOT], bf16, kind="ExternalInput")
    outd = nc.dram_tensor("out", [NI, J], f32, kind="ExternalOutput")

    RELU = mybir.ActivationFunctionType.Relu
    IDENT = mybir.ActivationFunctionType.Identity
    ADD = mybir.AluOpType.add
    MAX = mybir.AluOpType.max

    with tile.TileContext(nc) as tc:
        with (
            tc.tile_pool(name="const", bufs=1) as cpool,
            tc.tile_pool(name="work", bufs=8) as work,
            tc.tile_pool(name="h2p", bufs=6) as h2pool,
            tc.tile_pool(name="h3p", bufs=2) as h3pool,
            tc.tile_pool(name="ps2", bufs=2, space="PSUM") as ps2,
            tc.tile_pool(name="ps3", bufs=1, space="PSUM") as ps3,
            tc.tile_pool(name="pso", bufs=1, space="PSUM") as pso,
        ):
            fin = cpool.tile([D, F_TOT], f32)
            hin = cpool.tile([D, H_TOT], bf16)
            xab = cpool.tile([D, NI], f32)
            ybT = cpool.tile([D, J], bf16)
            xTb = cpool.tile([D, NI], bf16)

            nc.sync.dma_start(fin[:], f32in[:])
            nc.sync.dma_start(hin[:], bf16in[:])

            W2_sb = hin[:, H_W2 : H_W2 + 64]
            b1_sb = fin[:, F_B1 : F_B1 + 1]
            b2_sb = fin[:, F_B2 : F_B2 + 1]
            b3_sb = fin[:, F_B3 : F_B3 + 1]
            b4_sb = fin[:, F_B4 : F_B4 + 1]

            # bf16 casts of xT / yT for the layer-1 matmuls
            nc.vector.tensor_copy(xTb[:], fin[:, F_XT : F_XT + NI])
            yTb = cpool.tile([D, J], bf16)
            nc.vector.tensor_copy(yTb[:], fin[:, F_YT : F_YT + J])

            # ---- precompute: xab = W1a^T @ xT + b1 ; ybT = W1b^T @ yT ----
            xa_ps = ps2.tile([D, J], mybir.dt.float32, tag="ps2")
            for c in range(2):
                nc.tensor.matmul(
                    xa_ps[64 * c : 64 * c + 64, :NI],
                    hin[:, H_W1A + 64 * c : H_W1A + 64 * c + 64],
                    xTb[:],
                    tile_position=(0, 64 * c),
                )
            nc.scalar.activation(xab[:], xa_ps[:, :NI], IDENT, bias=b1_sb)

            yb_ps = ps2.tile([D, J], mybir.dt.float32, tag="ps2")
            for c in range(2):
                for jc in range(2):
                    nc.tensor.matmul(
                        yb_ps[64 * c : 64 * c + 64, JC * jc : JC * jc + JC],
                        hin[:, H_W1B + 64 * c : H_W1B + 64 * c + 64],
                        yTb[:, JC * jc : JC * jc + JC],
                        tile_position=(0, 64 * c),
                    )
            nc.scalar.activation(ybT[:], yb_ps[:], IDENT)

            # ---- main loop (software-pipelined emission) ----
            # h1 production runs LOOKAHEAD pairs ahead of consumption so a
            # DVE-assigned h2 evacuation (which waits on PE matmuls) never
            # starves the PE of h1 inputs queued behind it. The L4 matmul of
            # group g is deferred into group g+1's pair loop so the PE's
            # wait on relu-h3 (ScalarE) doesn't block the next group's L2s.
            out_ps = pso.tile([D, J], mybir.dt.float32)
            LOOKAHEAD = 2
            NPAIR = 64

            def x_col(gp):
                g, p = gp // 8, gp % 8
                return 64 * (g // 4) + 16 * (g % 4) + 2 * p

            def emit_h1(gp):
                h1e = work.tile([D, J], bf16, tag="h1")
                h1o = work.tile([D, J], bf16, tag="h1")
                ie = x_col(gp)
                nc.vector.tensor_scalar(
                    h1e[:], ybT[:], xab[:, ie : ie + 1], 0.0, ADD, MAX
                )
                nc.vector.tensor_scalar(
                    h1o[:], ybT[:], xab[:, ie + 1 : ie + 2], 0.0, ADD, MAX
                )
                return h1e, h1o

            def _emit_l4(g, h3pack):
                t4, v4 = g // 4, g % 4
                for jc in range(2):
                    jsl = slice(JC * jc, JC * jc + JC)
                    nc.tensor.matmul(
                        out_ps[64 * t4 : 64 * t4 + 64, jsl],
                        hin[:, H_W4P + 64 * v4 : H_W4P + 64 * v4 + 64],
                        h3pack[:, jsl],
                        tile_position=(0, 64 * t4),
                        start=(v4 == 0),
                        stop=(v4 == 3),
                    )

            h1q = {gp: emit_h1(gp) for gp in range(LOOKAHEAD)}
            ps3_t = None
            pend_l4 = None  # (g, h3pack) awaiting L4 emission

            for g in range(8):
                ps3_t = ps3.tile([D, J], mybir.dt.float32, tag="ps3")
                for p in range(8):
                    gp = g * 8 + p
                    t, v = p // 4, p % 4
                    if gp + LOOKAHEAD < NPAIR:
                        h1q[gp + LOOKAHEAD] = emit_h1(gp + LOOKAHEAD)
                    h1e, h1o = h1q.pop(gp)
                    ps2_t = ps2.tile([D, J], mybir.dt.float32, tag="ps2")
                    for jc in range(2):
                        jsl = slice(JC * jc, JC * jc + JC)
                        nc.tensor.matmul(
                            ps2_t[0:64, jsl], W2_sb, h1e[:, jsl],
                            tile_position=(0, 0),
                        )
                        nc.tensor.matmul(
                            ps2_t[64:128, jsl], W2_sb, h1o[:, jsl],
                            tile_position=(0, 64),
                        )
                    if pend_l4 is not None and p == 1:
                        _emit_l4(*pend_l4)
                        pend_l4 = None
                    h2pack = h2pool.tile([D, J], bf16, tag="h2")
                    if gp % EVAC_DVE_MOD == EVAC_DVE_PHASE:
                        nc.vector.tensor_scalar(
                            h2pack[:], ps2_t[:], b2_sb, 0.0, ADD, MAX
                        )
                    else:
                        nc.scalar.activation(
                            h2pack[:], ps2_t[:], RELU, bias=b2_sb
                        )
                    for jc in range(2):
                        jsl = slice(JC * jc, JC * jc + JC)
                        nc.tensor.matmul(
                            ps3_t[64 * t : 64 * t + 64, jsl],
                            hin[:, H_W3P + 64 * v : H_W3P + 64 * v + 64],
                            h2pack[:, jsl],
                            tile_position=(0, 64 * t),
                            start=(v == 0),
                            stop=(v == 3),
                        )
                h3pack = h3pool.tile([D, J], bf16, tag="h3")
                nc.scalar.activation(h3pack[:], ps3_t[:], RELU, bias=b3_sb)
                pend_l4 = (g, h3pack)

            _emit_l4(*pend_l4)
            pend_l4 = None

            out_sb = cpool.tile([NI, J], f32)
            nc.scalar.activation(out_sb[:], out_ps[:], IDENT, bias=b4_sb)
            nc.sync.dma_start(outd[:], out_sb[:])

    nc.compile()
    return nc


def _get_compiled():
    if "nc" not in _CACHE:
        _CACHE["nc"] = _build_bass()
    return _CACHE["nc"]


def _prep_in_maps(x, y, W1, b1, W2, b2, W3, b3, W4, b4):
    d = x.shape[1]
    W1a = W1[:d]
    W1b = W1[d:]
    W3P, W4P = _build_packed_weights(W3, W4)

    f32pack = np.empty((D, F_TOT), np.float32)
    f32pack[:, F_YT : F_YT + J] = y.T
    f32pack[:, F_B1] = b1
    f32pack[:, F_B2] = np.concatenate([b2, b2])
    f32pack[:, F_B3] = np.tile(b3, 16)
    f32pack[:, F_B4] = b4[0]

    bfpack = np.empty((D, H_TOT), BF16)
    bfpack[:, H_W1A : H_W1A + D] = W1a.astype(BF16)
    bfpack[:, H_W1B : H_W1B + D] = W1b.astype(BF16)
    bfpack[:, H_W2 : H_W2 + 64] = W2.astype(BF16)
    bfpack[:, H_W3P : H_W3P + 256] = (
        W3P.transpose(1, 0, 2).reshape(D, 256).astype(BF16)
    )
    bfpack[:, H_W4P : H_W4P + 256] = (
        W4P.transpose(1, 0, 2).reshape(D, 256).astype(BF16)
    )
    bfpack = np.ascontiguousarray(bfpack)

    in_maps = []
    for c in range(N_CORES):
        fp = f32pack.copy()
        fp[:, F_XT : F_XT + NI] = x[c * NI : (c + 1) * NI].T
        in_maps.append({"f32in": fp, "bf16in": bfpack})
    return in_maps


def run(x, y, W1, b1, W2, b2, W3, b3, W4, b4, **spmd_kwargs):
    """Run the kernel, returning (output, BassKernelResults)."""
    args = [np.asarray(a, np.float32) for a in
            (x, y, W1, b1, W2, b2, W3, b3, W4, b4)]
    in_maps = _prep_in_maps(*args)
    nc = _get_compiled()
    res = run_bass_kernel_spmd(nc, in_maps, list(range(N_CORES)), **spmd_kwargs)
    out = np.concatenate([np.asarray(r["out"]) for r in res.results], axis=0)
    return out.astype(np.float32), res


def kernel(x, y, W1, b1, W2, b2, W3, b3, W4, b4):
    out, _ = run(x, y, W1, b1, W2, b2, W3, b3, W4, b4)
    return out
